# revision 8
# baseline (speedup 1.0000x reference)
"""Grouped-channel attention (CAT FullAttention) Trainium2 kernel.

Math (per batch element b; L=S=96, R=70, E=10, P=7):
  scores[l,s,p,r] = sum_e q[l,e,p] * k[s,e,r]
  A = softmax over (s,p) of scores           (per l, r)
  out[l,e,r]      = sum_{s,p} v[s,e,p] * A[l,s,p,r]

Strategy: pure data parallel over the batch dim (B=256 -> 32 per core x 8
cores). Per batch element on-device:
  e1   (PE) : per r, scores[s,(p,l)] = K_r^T @ Q2, fp32r matmuls, N=336
              chunks written to 512-element-aligned PSUM slots.
  exp  (ACT): exp over 3 PSUM slots per instr -> A [96, 4704] in SBUF.
  e2   (PE) : per p, E[e',(r,l)] += V_p^T @ A_p accumulated over p in PSUM.
              V carries a ones-channel at e'=10, so E[10,:] is the softmax
              denominator.
  tail (PE/DVE): transpose E to [l,(r,e')], reciprocal of the denominator,
              broadcast multiply, contiguous DMA of [96,70] per batch.
"""

import sys

if "/opt/trn_rl_repo" not in sys.path:
    sys.path.insert(0, "/opt/trn_rl_repo")

import numpy as np

import concourse.bass as bass
import concourse.bacc as bacc
import concourse.tile as tile
from concourse import mybir, masks
from concourse.bass_utils import run_bass_kernel_spmd

B, L, R = 256, 96, 70
E, P = 10, 7
EP = E + 1  # v channels + ones channel
NCORES = 8
BPC = B // NCORES  # batches per core
G = 4  # batches per DMA group
F32R = mybir.dt.float32r
F32 = mybir.dt.float32

_CACHE = {}


def _build(bpc, repeat=1):
    nc = bacc.Bacc("TRN2", target_bir_lowering=False, debug=False, num_devices=NCORES)
    q_d = nc.dram_tensor("q2", [bpc, E, P * L], F32R, kind="ExternalInput").ap()
    k_d = nc.dram_tensor("kt", [bpc, E, P * L], F32R, kind="ExternalInput").ap()
    v_d = nc.dram_tensor("vt", [bpc, L, P * EP], F32R, kind="ExternalInput").ap()
    o_d = nc.dram_tensor("out", [bpc, L, R], F32, kind="ExternalOutput").ap()

    ngroups = bpc // G
    CH = 336  # e1 chunk width: (p,l)=672 split in two, each >=256 for fp32r
    SLOT = 512  # psum chunk slot (one bank)
    NCHUNK = 2 * P  # 14 chunks of 336 per batch
    # chunk-tile packing: 3 slots per scores tile -> 5 ACT instrs per batch
    TILES = [(0, 3), (3, 3), (6, 3), (9, 3), (12, 2)]

    with tile.TileContext(nc) as tc:
        with (
            tc.tile_pool(name="const", bufs=1) as cpool,
            tc.tile_pool(name="qk", bufs=2) as qkpool,
            tc.tile_pool(name="apool", bufs=3) as apool,
            tc.tile_pool(name="esb", bufs=2) as epool,
            tc.tile_pool(name="rd", bufs=2) as rdpool,
            tc.tile_pool(name="og", bufs=2) as ogpool,
            tc.tile_pool(name="ps_s", bufs=2, space="PSUM") as spool,
            tc.tile_pool(name="ps_e", bufs=1, space="PSUM") as e2pool,
        ):
            ident = cpool.tile([EP, EP], F32)
            masks.make_identity(nc, ident[:])
            # touch Exp once so the ACT table set loads during the first DMA
            warm = cpool.tile([1, 1], F32)
            nc.scalar.activation(
                warm[:], ident[0:1, 0:1], mybir.ActivationFunctionType.Exp
            )

            # per-group input tiles, per-batch A tiles
            qg = [None] * ngroups
            kg = [None] * ngroups
            vg = [None] * ngroups
            A = [None] * bpc
            Et = [None] * bpc
            Eb = [None] * bpc
            OG = [None] * ngroups

            def load_group(g):
                qt = qkpool.tile([E, G * P * L], F32R, tag="qg")
                kt = qkpool.tile([E, G * P * L], F32R, tag="kg")
                vt = qkpool.tile([L, G * P * EP], F32R, tag="vg")
                g0 = g * G
                nc.sync.dma_start(
                    qt[:].rearrange("e (b f) -> e b f", b=G),
                    q_d[g0 : g0 + G].rearrange("b e f -> e b f"),
                )
                nc.sync.dma_start(
                    kt[:].rearrange("e (b f) -> e b f", b=G),
                    k_d[g0 : g0 + G].rearrange("b e f -> e b f"),
                )
                nc.sync.dma_start(
                    vt[:].rearrange("s (b f) -> s b f", b=G),
                    v_d[g0 : g0 + G].rearrange("b s f -> s b f"),
                )
                qg[g], kg[g], vg[g] = qt, kt, vt

            def stage1(b):
                """e1 matmuls + exp into A[b]."""
                g, i = divmod(b, G)
                qt, kt = qg[g], kg[g]
                at = apool.tile([L, P * P * L], F32R, tag="A")
                A[b] = at
                for j0, nch in TILES:
                    st = spool.tile([L, 3 * SLOT], F32, tag="s")
                    for m in range(nch):
                        jj = j0 + m
                        r, c = divmod(jj, 2)
                        nc.tensor.matmul(
                            st[:, m * SLOT : m * SLOT + CH],
                            lhsT=kt[:, i * 672 + r * L : i * 672 + (r + 1) * L],
                            rhs=qt[:, i * 672 + c * CH : i * 672 + (c + 1) * CH],
                            start=True,
                            stop=True,
                        )
                    src = st[:, 0 : nch * SLOT].rearrange(
                        "s (a c) -> s a c", c=SLOT
                    )[:, :, 0:CH]
                    dst = at[:, j0 * CH : (j0 + nch) * CH].rearrange(
                        "s (a c) -> s a c", c=CH
                    )
                    nc.scalar.activation(dst, src, mybir.ActivationFunctionType.Exp)

            def stage2(b):
                """e2 accumulation, evacuate, transpose, normalize."""
                g, i = divmod(b, G)
                vt = vg[g]
                at = A[b]
                a3 = at[:].rearrange("s (r f) -> s r f", f=P * L)
                et = e2pool.tile([EP, 2 * SLOT], F32, tag="e2")
                Et[b] = et
                o0 = et[:, 0:384].rearrange("e (r l) -> e r l", l=L)
                o1 = et[:, SLOT : SLOT + 288].rearrange("e (r l) -> e r l", l=L)
                for p in range(P):
                    lhsT = vt[:, i * 77 + p * EP : i * 77 + (p + 1) * EP]
                    nc.tensor.matmul(
                        o0,
                        lhsT=lhsT,
                        rhs=a3[:, 0:4, p * L : (p + 1) * L],
                        start=(p == 0),
                        stop=(p == P - 1),
                    )
                    nc.tensor.matmul(
                        o1,
                        lhsT=lhsT,
                        rhs=a3[:, 4:7, p * L : (p + 1) * L],
                        start=(p == 0),
                        stop=(p == P - 1),
                    )

            def stage3(b):
                g, i = divmod(b, G)
                et = Et[b]
                eb = epool.tile([EP, P * L], F32, tag="eb")
                Eb[b] = eb
                nc.vector.tensor_copy(eb[:, 0:384], et[:, 0:384])
                nc.vector.tensor_copy(eb[:, 384:672], et[:, SLOT : SLOT + 288])
                tt = e2pool.tile([L, P * EP], F32, tag="e2")
                for r in range(P):
                    nc.tensor.transpose(
                        tt[:, r * EP : (r + 1) * EP],
                        eb[:, r * L : (r + 1) * L],
                        ident[:],
                    )
                t3 = tt[:].rearrange("l (r e) -> l r e", e=EP)
                rd = rdpool.tile([L, P], F32, tag="rd")
                r3 = rd[:].rearrange("l (r u) -> l r u", u=1)
                nc.vector.reciprocal(r3, t3[:, :, E : E + 1])
                if OG[g] is None:
                    og_tile = ogpool.tile([L, G * R], F32, tag="og")
                    OG[g] = og_tile
                og = OG[g]
                dst = og[:, i * R : (i + 1) * R].rearrange("l (e r) -> l r e", r=P)
                rdb = r3.copy()
                rdb.ap = rdb.ap[:-1] + [[0, E]]
                nc.vector.tensor_mul(dst, t3[:, :, 0:E], rdb)

            def flush_group(g):
                g0 = g * G
                nc.sync.dma_start(
                    o_d[g0 : g0 + G].rearrange("b l c -> l b c"),
                    OG[g][:].rearrange("l (b c) -> l b c", b=G),
                )
                OG[g] = None

            # software pipeline: stage1(b) ahead, stage2/3 one batch behind
            for rep in range(repeat):
                load_group(0)
                for b in range(bpc + 1):
                    if b < bpc:
                        g, i = divmod(b, G)
                        if i == 0 and g + 1 < ngroups:
                            load_group(g + 1)
                        stage1(b)
                    if b >= 1:
                        stage2(b - 1)
                        stage3(b - 1)
                        if (b - 1) % G == G - 1:
                            flush_group((b - 1) // G)

    nc.compile()
    return nc


def _get_nc(bpc=BPC, repeat=1):
    key = (bpc, repeat)
    if key not in _CACHE:
        _CACHE[key] = _build(bpc, repeat)
    return _CACHE[key]


def _prep(queries, keys, values):
    q = np.asarray(queries, dtype=np.float32)
    k = np.asarray(keys, dtype=np.float32)
    v = np.asarray(values, dtype=np.float32)
    b = q.shape[0]
    # Q2[b, e, p*96+l] = q[b, l, e*7+p]
    q2 = np.ascontiguousarray(
        q.reshape(b, L, E, P).transpose(0, 2, 3, 1).reshape(b, E, P * L)
    )
    # KT[b, e, r*96+s] = k[b, s, e*7+r]
    kt = np.ascontiguousarray(
        k.reshape(b, L, E, P).transpose(0, 2, 3, 1).reshape(b, E, P * L)
    )
    # VT[b, s, p*11+e'] = v[b, s, e'*7+p] for e'<10, 1.0 at e'=10
    v4 = v.reshape(b, L, E, P).transpose(0, 1, 3, 2)  # [b, s, p, e]
    vt = np.concatenate([v4, np.ones((b, L, P, 1), np.float32)], axis=-1)
    vt = np.ascontiguousarray(vt.reshape(b, L, P * EP))
    return q2, kt, vt


def kernel(queries, keys, values, attn_mask=None, _trace=False):
    nc = _get_nc()
    q2, kt, vt = _prep(queries, keys, values)
    in_maps = []
    for c in range(NCORES):
        s = slice(c * BPC, (c + 1) * BPC)
        in_maps.append({"q2": q2[s], "kt": kt[s], "vt": vt[s]})
    res = None
    for attempt in range(3):
        try:
            res = run_bass_kernel_spmd(
                nc, in_maps, core_ids=list(range(NCORES)), trace=_trace
            )
            break
        except Exception:
            # shared terminal occasionally reports transient NRT device
            # errors; back off and retry
            if attempt == 2:
                raise
            import time as _time

            _time.sleep(15)
    out = np.concatenate([res.results[c]["out"] for c in range(NCORES)], axis=0)
    if _trace:
        kernel.last_exec_time_ns = res.exec_time_ns
        kernel.last_results = res
    return out.astype(np.float32)


# revision 22
# speedup vs baseline: 1.0048x; 1.0048x over previous
"""Grouped-channel attention (CAT FullAttention) Trainium2 kernel.

Math (per batch element b; L=S=96, R=70, E=10, P=7):
  scores[l,s,p,r] = sum_e q[l,e,p] * k[s,e,r]
  A = softmax over (s,p) of scores           (per l, r)
  out[l,e,r]      = sum_{s,p} v[s,e,p] * A[l,s,p,r]

Strategy: pure data parallel over the batch dim (B=256 -> 32 per core x 8
cores). Per batch element on-device:
  e1   (PE) : per r, scores[s,(p,l)] = K_r^T @ Q2, fp32r matmuls, N=336
              chunks written to 512-element-aligned PSUM slots.
  exp  (ACT): exp over 3 PSUM slots per instr -> A [96, 4704] in SBUF.
  e2   (PE) : per p, E[e',(r,l)] += V_p^T @ A_p accumulated over p in PSUM.
              V carries a ones-channel at e'=10, so E[10,:] is the softmax
              denominator.
  tail (PE/DVE): transpose E to [l,(r,e')], reciprocal of the denominator,
              broadcast multiply, contiguous DMA of [96,70] per batch.
"""

import sys

if "/opt/trn_rl_repo" not in sys.path:
    sys.path.insert(0, "/opt/trn_rl_repo")

import numpy as np

import concourse.bass as bass
import concourse.bacc as bacc
import concourse.tile as tile
from concourse import mybir, masks
from concourse.bass_utils import run_bass_kernel_spmd

B, L, R = 256, 96, 70
E, P = 10, 7
EP = E + 1  # v channels + ones channel
NCORES = 8
BPC = B // NCORES  # batches per core
G = 4  # batches per DMA group
F32R = mybir.dt.float32r
F32 = mybir.dt.float32

_CACHE = {}


def _build(bpc, repeat=1):
    nc = bacc.Bacc("TRN2", target_bir_lowering=False, debug=False, num_devices=NCORES)
    q_d = nc.dram_tensor("q2", [bpc, E, P * L], F32R, kind="ExternalInput").ap()
    k_d = nc.dram_tensor("kt", [bpc, E, P * L], F32R, kind="ExternalInput").ap()
    v_d = nc.dram_tensor("vt", [bpc, L, P * EP], F32R, kind="ExternalInput").ap()
    o_d = nc.dram_tensor("out", [bpc, L, R], F32, kind="ExternalOutput").ap()

    ngroups = bpc // G
    CH = 336  # e1 chunk width: (p,l)=672 split in two, each >=256 for fp32r
    SLOT = 512  # psum chunk slot (one bank)
    NCHUNK = 2 * P  # 14 chunks of 336 per batch
    # chunk-tile packing: 3 slots per scores tile -> 5 ACT instrs per batch
    TILES = [(0, 3), (3, 3), (6, 3), (9, 3), (12, 2)]

    with tile.TileContext(nc) as tc:
        with (
            tc.tile_pool(name="const", bufs=1) as cpool,
            tc.tile_pool(name="qk", bufs=2) as qkpool,
            tc.tile_pool(name="apool", bufs=3) as apool,
            tc.tile_pool(name="esb", bufs=2) as epool,
            tc.tile_pool(name="rd", bufs=2) as rdpool,
            tc.tile_pool(name="og", bufs=2) as ogpool,
            tc.tile_pool(name="ps_s", bufs=2, space="PSUM") as spool,
            tc.tile_pool(name="ps_e", bufs=1, space="PSUM") as e2pool,
        ):
            ident = cpool.tile([EP, EP], F32)
            masks.make_identity(nc, ident[:])
            # touch Exp once so the ACT table set loads during the first DMA
            warm = cpool.tile([1, 1], F32)
            nc.scalar.activation(
                warm[:], ident[0:1, 0:1], mybir.ActivationFunctionType.Exp
            )

            # per-group input tiles, per-batch A tiles
            qg = [None] * ngroups
            kg = [None] * ngroups
            vg = [None] * ngroups
            A = [None] * bpc
            Et = [None] * bpc
            Eb = [None] * bpc
            OG = [None] * ngroups

            def load_group(g):
                qt = qkpool.tile([E, G * P * L], F32R, tag="qg")
                kt = qkpool.tile([E, G * P * L], F32R, tag="kg")
                vt = qkpool.tile([L, G * P * EP], F32R, tag="vg")
                g0 = g * G
                if g == 0:
                    # split the first group's q/k so batch 0 is ready early
                    nc.sync.dma_start(qt[:, 0:672], q_d[g0])
                    nc.sync.dma_start(kt[:, 0:672], k_d[g0])
                    nc.sync.dma_start(
                        qt[:, 672:].rearrange("e (b f) -> e b f", b=G - 1),
                        q_d[g0 + 1 : g0 + G].rearrange("b e f -> e b f"),
                    )
                    nc.sync.dma_start(
                        kt[:, 672:].rearrange("e (b f) -> e b f", b=G - 1),
                        k_d[g0 + 1 : g0 + G].rearrange("b e f -> e b f"),
                    )
                else:
                    nc.sync.dma_start(
                        qt[:].rearrange("e (b f) -> e b f", b=G),
                        q_d[g0 : g0 + G].rearrange("b e f -> e b f"),
                    )
                    nc.sync.dma_start(
                        kt[:].rearrange("e (b f) -> e b f", b=G),
                        k_d[g0 : g0 + G].rearrange("b e f -> e b f"),
                    )
                nc.sync.dma_start(
                    vt[:].rearrange("s (b f) -> s b f", b=G),
                    v_d[g0 : g0 + G].rearrange("b s f -> s b f"),
                )
                qg[g], kg[g], vg[g] = qt, kt, vt

            def stage1(b):
                """e1 matmuls + exp into A[b]."""
                g, i = divmod(b, G)
                qt, kt = qg[g], kg[g]
                at = apool.tile([L, P * P * L], F32R, tag="A")
                A[b] = at
                for j0, nch in TILES:
                    st = spool.tile([L, 3 * SLOT], F32, tag="s")
                    for m in range(nch):
                        jj = j0 + m
                        r, c = divmod(jj, 2)
                        nc.tensor.matmul(
                            st[:, m * SLOT : m * SLOT + CH],
                            lhsT=kt[:, i * 672 + r * L : i * 672 + (r + 1) * L],
                            rhs=qt[:, i * 672 + c * CH : i * 672 + (c + 1) * CH],
                            start=True,
                            stop=True,
                        )
                    src = st[:, 0 : nch * SLOT].rearrange(
                        "s (a c) -> s a c", c=SLOT
                    )[:, :, 0:CH]
                    dst = at[:, j0 * CH : (j0 + nch) * CH].rearrange(
                        "s (a c) -> s a c", c=CH
                    )
                    nc.scalar.activation(dst, src, mybir.ActivationFunctionType.Exp)

            def stage2(b):
                """e2 accumulation, evacuate, transpose, normalize."""
                g, i = divmod(b, G)
                vt = vg[g]
                at = A[b]
                a3 = at[:].rearrange("s (r f) -> s r f", f=P * L)
                et = e2pool.tile([EP, 2 * SLOT], F32, tag="e2")
                Et[b] = et
                o0 = et[:, 0:384].rearrange("e (r l) -> e r l", l=L)
                o1 = et[:, SLOT : SLOT + 288].rearrange("e (r l) -> e r l", l=L)
                for p in range(P):
                    lhsT = vt[:, i * 77 + p * EP : i * 77 + (p + 1) * EP]
                    nc.tensor.matmul(
                        o0,
                        lhsT=lhsT,
                        rhs=a3[:, 0:4, p * L : (p + 1) * L],
                        start=(p == 0),
                        stop=(p == P - 1),
                    )
                    nc.tensor.matmul(
                        o1,
                        lhsT=lhsT,
                        rhs=a3[:, 4:7, p * L : (p + 1) * L],
                        start=(p == 0),
                        stop=(p == P - 1),
                    )

            def stage3(b):
                g, i = divmod(b, G)
                et = Et[b]
                eb = epool.tile([EP, P * L], F32, tag="eb")
                Eb[b] = eb
                nc.vector.tensor_copy(eb[:, 0:384], et[:, 0:384])
                nc.vector.tensor_copy(eb[:, 384:672], et[:, SLOT : SLOT + 288])
                if b >= bpc - 2:
                    # scores pool is idle for the final batches; keep the
                    # e2 accumulator slot free so the last e2 starts sooner
                    tt = spool.tile([L, P * EP], F32, tag="s")
                else:
                    tt = e2pool.tile([L, P * EP], F32, tag="e2")
                for r in range(P):
                    nc.tensor.transpose(
                        tt[:, r * EP : (r + 1) * EP],
                        eb[:, r * L : (r + 1) * L],
                        ident[:],
                    )
                t3 = tt[:].rearrange("l (r e) -> l r e", e=EP)
                rd = rdpool.tile([L, P], F32, tag="rd")
                r3 = rd[:].rearrange("l (r u) -> l r u", u=1)
                nc.vector.reciprocal(r3, t3[:, :, E : E + 1])
                if OG[g] is None:
                    og_tile = ogpool.tile([L, G * R], F32, tag="og")
                    OG[g] = og_tile
                og = OG[g]
                dst = og[:, i * R : (i + 1) * R].rearrange("l (e r) -> l r e", r=P)
                rdb = r3.copy()
                rdb.ap = rdb.ap[:-1] + [[0, E]]
                nc.vector.tensor_mul(dst, t3[:, :, 0:E], rdb)

            def flush_group(g):
                g0 = g * G
                nc.sync.dma_start(
                    o_d[g0 : g0 + G].rearrange("b l c -> l b c"),
                    OG[g][:].rearrange("l (b c) -> l b c", b=G),
                )
                OG[g] = None

            # software pipeline: stage1(b) ahead, stage2/3 one batch behind
            for rep in range(repeat):
                load_group(0)
                for b in range(bpc + 1):
                    if b < bpc:
                        g, i = divmod(b, G)
                        if i == 0 and g + 1 < ngroups:
                            load_group(g + 1)
                        stage1(b)
                    if b >= 1:
                        stage2(b - 1)
                        stage3(b - 1)
                        if (b - 1) % G == G - 1:
                            flush_group((b - 1) // G)

    nc.compile()
    return nc


def _get_nc(bpc=BPC, repeat=1):
    key = (bpc, repeat)
    if key not in _CACHE:
        _CACHE[key] = _build(bpc, repeat)
    return _CACHE[key]


def _prep(queries, keys, values):
    q = np.asarray(queries, dtype=np.float32)
    k = np.asarray(keys, dtype=np.float32)
    v = np.asarray(values, dtype=np.float32)
    b = q.shape[0]
    # Q2[b, e, p*96+l] = q[b, l, e*7+p]
    q2 = np.ascontiguousarray(
        q.reshape(b, L, E, P).transpose(0, 2, 3, 1).reshape(b, E, P * L)
    )
    # KT[b, e, r*96+s] = k[b, s, e*7+r]
    kt = np.ascontiguousarray(
        k.reshape(b, L, E, P).transpose(0, 2, 3, 1).reshape(b, E, P * L)
    )
    # VT[b, s, p*11+e'] = v[b, s, e'*7+p] for e'<10, 1.0 at e'=10
    v4 = v.reshape(b, L, E, P).transpose(0, 1, 3, 2)  # [b, s, p, e]
    vt = np.concatenate([v4, np.ones((b, L, P, 1), np.float32)], axis=-1)
    vt = np.ascontiguousarray(vt.reshape(b, L, P * EP))
    return q2, kt, vt


def kernel(queries, keys, values, attn_mask=None, _trace=False):
    nc = _get_nc()
    q2, kt, vt = _prep(queries, keys, values)
    in_maps = []
    for c in range(NCORES):
        s = slice(c * BPC, (c + 1) * BPC)
        in_maps.append({"q2": q2[s], "kt": kt[s], "vt": vt[s]})
    res = None
    for attempt in range(3):
        try:
            res = run_bass_kernel_spmd(
                nc, in_maps, core_ids=list(range(NCORES)), trace=_trace
            )
            break
        except Exception:
            # shared terminal occasionally reports transient NRT device
            # errors; back off and retry
            if attempt == 2:
                raise
            import time as _time

            _time.sleep(15)
    out = np.concatenate([res.results[c]["out"] for c in range(NCORES)], axis=0)
    if _trace:
        kernel.last_exec_time_ns = res.exec_time_ns
        kernel.last_results = res
    return out.astype(np.float32)


# revision 31
# speedup vs baseline: 1.0131x; 1.0083x over previous
"""Grouped-channel attention (CAT FullAttention) Trainium2 kernel.

Math (per batch element b; L=S=96, R=70, E=10, P=7):
  scores[l,s,p,r] = sum_e q[l,e,p] * k[s,e,r]
  A = softmax over (s,p) of scores           (per l, r)
  out[l,e,r]      = sum_{s,p} v[s,e,p] * A[l,s,p,r]

Strategy: pure data parallel over the batch dim (B=256 -> 32 per core x 8
cores). Per batch element on-device:
  e1   (PE) : per r, scores[s,(p,l)] = K_r^T @ Q2, fp32r matmuls, N=336
              chunks written to 512-element-aligned PSUM slots.
  exp  (ACT): exp over 3 PSUM slots per instr -> A [96, 4704] in SBUF.
  e2   (PE) : per p, E[e',(r,l)] += V_p^T @ A_p accumulated over p in PSUM.
              V carries a ones-channel at e'=10, so E[10,:] is the softmax
              denominator.
  tail (PE/DVE): transpose E to [l,(r,e')], reciprocal of the denominator,
              broadcast multiply, contiguous DMA of [96,70] per batch.
"""

import sys

if "/opt/trn_rl_repo" not in sys.path:
    sys.path.insert(0, "/opt/trn_rl_repo")

import numpy as np

import concourse.bass as bass
import concourse.bacc as bacc
import concourse.tile as tile
from concourse import mybir, masks
from concourse.bass_utils import run_bass_kernel_spmd

B, L, R = 256, 96, 70
E, P = 10, 7
EP = E + 1  # v channels + ones channel
NCORES = 8
BPC = B // NCORES  # batches per core
G = 4  # batches per DMA group
F32R = mybir.dt.float32r
F32 = mybir.dt.float32

_CACHE = {}


def _build(bpc, repeat=1):
    nc = bacc.Bacc("TRN2", target_bir_lowering=False, debug=False, num_devices=NCORES)
    q_d = nc.dram_tensor("q2", [bpc, E, P * L], F32R, kind="ExternalInput").ap()
    k_d = nc.dram_tensor("kt", [bpc, E, P * L], F32R, kind="ExternalInput").ap()
    v_d = nc.dram_tensor("vt", [bpc, L, P * EP], F32R, kind="ExternalInput").ap()
    o_d = nc.dram_tensor("out", [bpc, L, R], F32, kind="ExternalOutput").ap()

    ngroups = bpc // G
    CH = 336  # e1 chunk width: (p,l)=672 split in two, each >=256 for fp32r
    SLOT = 512  # psum chunk slot (one bank)
    NCHUNK = 2 * P  # 14 chunks of 336 per batch
    # chunk-tile packing: 3 slots per scores tile -> 5 ACT instrs per batch
    TILES = [(0, 3), (3, 3), (6, 3), (9, 2), (11, 3)]

    with tile.TileContext(nc) as tc:
        with (
            tc.tile_pool(name="const", bufs=1) as cpool,
            tc.tile_pool(name="qk", bufs=2) as qkpool,
            tc.tile_pool(name="apool", bufs=3) as apool,
            tc.tile_pool(name="esb", bufs=2) as epool,
            tc.tile_pool(name="rd", bufs=2) as rdpool,
            tc.tile_pool(name="og", bufs=2) as ogpool,
            tc.tile_pool(name="ps_s", bufs=2, space="PSUM") as spool,
            tc.tile_pool(name="ps_e", bufs=1, space="PSUM") as e2pool,
        ):
            ident = cpool.tile([EP, EP], F32)
            masks.make_identity(nc, ident[:])
            # touch Exp once so the ACT table set loads during the first DMA
            warm = cpool.tile([1, 1], F32)
            nc.scalar.activation(
                warm[:], ident[0:1, 0:1], mybir.ActivationFunctionType.Exp
            )

            # per-group input tiles, per-batch A tiles
            qg = [None] * ngroups
            kg = [None] * ngroups
            vg = [None] * ngroups
            A = [None] * bpc
            Et = [None] * bpc
            Eb = [None] * bpc
            OG = [None] * ngroups

            def load_group(g):
                qt = qkpool.tile([E, G * P * L], F32R, tag="qg")
                kt = qkpool.tile([E, G * P * L], F32R, tag="kg")
                vt = qkpool.tile([L, G * P * EP], F32R, tag="vg")
                g0 = g * G
                if g == 0:
                    # split the first group's q/k so batch 0 is ready early
                    nc.sync.dma_start(qt[:, 0:672], q_d[g0])
                    nc.sync.dma_start(kt[:, 0:672], k_d[g0])
                    nc.sync.dma_start(
                        qt[:, 672:].rearrange("e (b f) -> e b f", b=G - 1),
                        q_d[g0 + 1 : g0 + G].rearrange("b e f -> e b f"),
                    )
                    nc.sync.dma_start(
                        kt[:, 672:].rearrange("e (b f) -> e b f", b=G - 1),
                        k_d[g0 + 1 : g0 + G].rearrange("b e f -> e b f"),
                    )
                else:
                    nc.sync.dma_start(
                        qt[:].rearrange("e (b f) -> e b f", b=G),
                        q_d[g0 : g0 + G].rearrange("b e f -> e b f"),
                    )
                    nc.sync.dma_start(
                        kt[:].rearrange("e (b f) -> e b f", b=G),
                        k_d[g0 : g0 + G].rearrange("b e f -> e b f"),
                    )
                nc.sync.dma_start(
                    vt[:].rearrange("s (b f) -> s b f", b=G),
                    v_d[g0 : g0 + G].rearrange("b s f -> s b f"),
                )
                qg[g], kg[g], vg[g] = qt, kt, vt

            def stage1(b):
                """e1 matmuls + exp into A[b]."""
                g, i = divmod(b, G)
                qt, kt = qg[g], kg[g]
                at = apool.tile([L, P * P * L], F32R, tag="A")
                A[b] = at
                for j0, nch in TILES:
                    st = spool.tile([L, 3 * SLOT], F32, tag="s")
                    for m in range(nch):
                        jj = j0 + m
                        r, c = divmod(jj, 2)
                        nc.tensor.matmul(
                            st[:, m * SLOT : m * SLOT + CH],
                            lhsT=kt[:, i * 672 + r * L : i * 672 + (r + 1) * L],
                            rhs=qt[:, i * 672 + c * CH : i * 672 + (c + 1) * CH],
                            start=True,
                            stop=True,
                        )
                    src = st[:, 0 : nch * SLOT].rearrange(
                        "s (a c) -> s a c", c=SLOT
                    )[:, :, 0:CH]
                    dst = at[:, j0 * CH : (j0 + nch) * CH].rearrange(
                        "s (a c) -> s a c", c=CH
                    )
                    nc.scalar.activation(dst, src, mybir.ActivationFunctionType.Exp)

            def stage2(b):
                """e2 accumulation, evacuate, transpose, normalize."""
                g, i = divmod(b, G)
                vt = vg[g]
                at = A[b]
                a3 = at[:].rearrange("s (r f) -> s r f", f=P * L)
                et = e2pool.tile([EP, 2 * SLOT], F32, tag="e2")
                Et[b] = et
                o0 = et[:, 0:384].rearrange("e (r l) -> e r l", l=L)
                o1 = et[:, SLOT : SLOT + 288].rearrange("e (r l) -> e r l", l=L)
                for p in range(P):
                    lhsT = vt[:, i * 77 + p * EP : i * 77 + (p + 1) * EP]
                    nc.tensor.matmul(
                        o0,
                        lhsT=lhsT,
                        rhs=a3[:, 0:4, p * L : (p + 1) * L],
                        start=(p == 0),
                        stop=(p == P - 1),
                    )
                    nc.tensor.matmul(
                        o1,
                        lhsT=lhsT,
                        rhs=a3[:, 4:7, p * L : (p + 1) * L],
                        start=(p == 0),
                        stop=(p == P - 1),
                    )

            def stage3(b):
                g, i = divmod(b, G)
                et = Et[b]
                eb = epool.tile([EP, P * L], F32, tag="eb")
                Eb[b] = eb
                nc.vector.tensor_copy(eb[:, 0:384], et[:, 0:384])
                nc.vector.tensor_copy(eb[:, 384:672], et[:, SLOT : SLOT + 288])
                if b >= bpc - 2:
                    # scores pool is idle for the final batches; keep the
                    # e2 accumulator slot free so the last e2 starts sooner
                    tt = spool.tile([L, P * EP], F32, tag="s")
                else:
                    tt = e2pool.tile([L, P * EP], F32, tag="e2")
                for r in range(P):
                    nc.tensor.transpose(
                        tt[:, r * EP : (r + 1) * EP],
                        eb[:, r * L : (r + 1) * L],
                        ident[:],
                    )
                t3 = tt[:].rearrange("l (r e) -> l r e", e=EP)
                rd = rdpool.tile([L, P], F32, tag="rd")
                r3 = rd[:].rearrange("l (r u) -> l r u", u=1)
                nc.vector.reciprocal(r3, t3[:, :, E : E + 1])
                if OG[g] is None:
                    og_tile = ogpool.tile([L, G * R], F32, tag="og")
                    OG[g] = og_tile
                og = OG[g]
                dst = og[:, i * R : (i + 1) * R].rearrange("l (e r) -> l r e", r=P)
                rdb = r3.copy()
                rdb.ap = rdb.ap[:-1] + [[0, E]]
                nc.vector.tensor_mul(dst, t3[:, :, 0:E], rdb)

            def flush_group(g):
                g0 = g * G
                nc.sync.dma_start(
                    o_d[g0 : g0 + G].rearrange("b l c -> l b c"),
                    OG[g][:].rearrange("l (b c) -> l b c", b=G),
                )
                OG[g] = None

            # software pipeline: stage1(b) ahead, stage2/3 one batch behind
            for rep in range(repeat):
                load_group(0)
                for b in range(bpc + 1):
                    if b < bpc:
                        g, i = divmod(b, G)
                        if i == 0 and g + 1 < ngroups:
                            load_group(g + 1)
                        stage1(b)
                    if b >= 1:
                        stage2(b - 1)
                        stage3(b - 1)
                        if (b - 1) % G == G - 1:
                            flush_group((b - 1) // G)

    nc.compile()
    return nc


def _get_nc(bpc=BPC, repeat=1):
    key = (bpc, repeat)
    if key not in _CACHE:
        _CACHE[key] = _build(bpc, repeat)
    return _CACHE[key]


def _prep(queries, keys, values):
    q = np.asarray(queries, dtype=np.float32)
    k = np.asarray(keys, dtype=np.float32)
    v = np.asarray(values, dtype=np.float32)
    b = q.shape[0]
    # Q2[b, e, p*96+l] = q[b, l, e*7+p]
    q2 = np.ascontiguousarray(
        q.reshape(b, L, E, P).transpose(0, 2, 3, 1).reshape(b, E, P * L)
    )
    # KT[b, e, r*96+s] = k[b, s, e*7+r]
    kt = np.ascontiguousarray(
        k.reshape(b, L, E, P).transpose(0, 2, 3, 1).reshape(b, E, P * L)
    )
    # VT[b, s, p*11+e'] = v[b, s, e'*7+p] for e'<10, 1.0 at e'=10
    v4 = v.reshape(b, L, E, P).transpose(0, 1, 3, 2)  # [b, s, p, e]
    vt = np.concatenate([v4, np.ones((b, L, P, 1), np.float32)], axis=-1)
    vt = np.ascontiguousarray(vt.reshape(b, L, P * EP))
    return q2, kt, vt


def kernel(queries, keys, values, attn_mask=None, _trace=False):
    nc = _get_nc()
    q2, kt, vt = _prep(queries, keys, values)
    in_maps = []
    for c in range(NCORES):
        s = slice(c * BPC, (c + 1) * BPC)
        in_maps.append({"q2": q2[s], "kt": kt[s], "vt": vt[s]})
    res = None
    for attempt in range(3):
        try:
            res = run_bass_kernel_spmd(
                nc, in_maps, core_ids=list(range(NCORES)), trace=_trace
            )
            break
        except Exception:
            # shared terminal occasionally reports transient NRT device
            # errors; back off and retry
            if attempt == 2:
                raise
            import time as _time

            _time.sleep(15)
    out = np.concatenate([res.results[c]["out"] for c in range(NCORES)], axis=0)
    if _trace:
        kernel.last_exec_time_ns = res.exec_time_ns
        kernel.last_results = res
    return out.astype(np.float32)


# revision 32
# speedup vs baseline: 1.0196x; 1.0064x over previous
"""Grouped-channel attention (CAT FullAttention) Trainium2 kernel.

Math (per batch element b; L=S=96, R=70, E=10, P=7):
  scores[l,s,p,r] = sum_e q[l,e,p] * k[s,e,r]
  A = softmax over (s,p) of scores           (per l, r)
  out[l,e,r]      = sum_{s,p} v[s,e,p] * A[l,s,p,r]

Strategy: pure data parallel over the batch dim (B=256 -> 32 per core x 8
cores). Per batch element on-device:
  e1   (PE) : per r, scores[s,(p,l)] = K_r^T @ Q2, fp32r matmuls, N=336
              chunks written to 512-element-aligned PSUM slots.
  exp  (ACT): exp over 3 PSUM slots per instr -> A [96, 4704] in SBUF.
  e2   (PE) : per p, E[e',(r,l)] += V_p^T @ A_p accumulated over p in PSUM.
              V carries a ones-channel at e'=10, so E[10,:] is the softmax
              denominator.
  tail (PE/DVE): transpose E to [l,(r,e')], reciprocal of the denominator,
              broadcast multiply, contiguous DMA of [96,70] per batch.
"""

import sys

if "/opt/trn_rl_repo" not in sys.path:
    sys.path.insert(0, "/opt/trn_rl_repo")

import numpy as np

import concourse.bass as bass
import concourse.bacc as bacc
import concourse.tile as tile
from concourse import mybir, masks
from concourse.bass_utils import run_bass_kernel_spmd

B, L, R = 256, 96, 70
E, P = 10, 7
EP = E + 1  # v channels + ones channel
NCORES = 8
BPC = B // NCORES  # batches per core
G = 4  # batches per DMA group
F32R = mybir.dt.float32r
F32 = mybir.dt.float32

_CACHE = {}


def _build(bpc, repeat=1):
    nc = bacc.Bacc("TRN2", target_bir_lowering=False, debug=False, num_devices=NCORES)
    q_d = nc.dram_tensor("q2", [bpc, E, P * L], F32R, kind="ExternalInput").ap()
    k_d = nc.dram_tensor("kt", [bpc, E, P * L], F32R, kind="ExternalInput").ap()
    v_d = nc.dram_tensor("vt", [bpc, L, P * EP], F32R, kind="ExternalInput").ap()
    o_d = nc.dram_tensor("out", [bpc, L, R], F32, kind="ExternalOutput").ap()

    ngroups = bpc // G
    CH = 336  # e1 chunk width: (p,l)=672 split in two, each >=256 for fp32r
    SLOT = 512  # psum chunk slot (one bank)
    NCHUNK = 2 * P  # 14 chunks of 336 per batch
    # chunk-tile packing: 3 slots per scores tile -> 5 ACT instrs per batch
    TILES = [(0, 3), (3, 3), (6, 3), (9, 2), (11, 3)]
    # batch 0 leads with a 1-slot region so the first exp fires one cold
    # matmul after the DMA instead of three
    TILES0 = [(0, 1), (1, 2), (3, 3), (6, 3), (9, 2), (11, 3)]

    with tile.TileContext(nc) as tc:
        with (
            tc.tile_pool(name="const", bufs=1) as cpool,
            tc.tile_pool(name="qk", bufs=2) as qkpool,
            tc.tile_pool(name="apool", bufs=3) as apool,
            tc.tile_pool(name="esb", bufs=2) as epool,
            tc.tile_pool(name="rd", bufs=2) as rdpool,
            tc.tile_pool(name="og", bufs=2) as ogpool,
            tc.tile_pool(name="ps_s", bufs=2, space="PSUM") as spool,
            tc.tile_pool(name="ps_e", bufs=1, space="PSUM") as e2pool,
        ):
            ident = cpool.tile([EP, EP], F32)
            masks.make_identity(nc, ident[:])
            # touch Exp once so the ACT table set loads during the first DMA
            warm = cpool.tile([1, 1], F32)
            nc.scalar.activation(
                warm[:], ident[0:1, 0:1], mybir.ActivationFunctionType.Exp
            )

            # per-group input tiles, per-batch A tiles
            qg = [None] * ngroups
            kg = [None] * ngroups
            vg = [None] * ngroups
            A = [None] * bpc
            Et = [None] * bpc
            Eb = [None] * bpc
            OG = [None] * ngroups

            def load_group(g):
                qt = qkpool.tile([E, G * P * L], F32R, tag="qg")
                kt = qkpool.tile([E, G * P * L], F32R, tag="kg")
                vt = qkpool.tile([L, G * P * EP], F32R, tag="vg")
                g0 = g * G
                if g == 0:
                    # split the first group's q/k so batch 0 is ready early
                    nc.sync.dma_start(qt[:, 0:672], q_d[g0])
                    nc.sync.dma_start(kt[:, 0:672], k_d[g0])
                    nc.sync.dma_start(
                        qt[:, 672:].rearrange("e (b f) -> e b f", b=G - 1),
                        q_d[g0 + 1 : g0 + G].rearrange("b e f -> e b f"),
                    )
                    nc.sync.dma_start(
                        kt[:, 672:].rearrange("e (b f) -> e b f", b=G - 1),
                        k_d[g0 + 1 : g0 + G].rearrange("b e f -> e b f"),
                    )
                else:
                    nc.sync.dma_start(
                        qt[:].rearrange("e (b f) -> e b f", b=G),
                        q_d[g0 : g0 + G].rearrange("b e f -> e b f"),
                    )
                    nc.sync.dma_start(
                        kt[:].rearrange("e (b f) -> e b f", b=G),
                        k_d[g0 : g0 + G].rearrange("b e f -> e b f"),
                    )
                nc.sync.dma_start(
                    vt[:].rearrange("s (b f) -> s b f", b=G),
                    v_d[g0 : g0 + G].rearrange("b s f -> s b f"),
                )
                qg[g], kg[g], vg[g] = qt, kt, vt

            def stage1(b):
                """e1 matmuls + exp into A[b]."""
                g, i = divmod(b, G)
                qt, kt = qg[g], kg[g]
                at = apool.tile([L, P * P * L], F32R, tag="A")
                A[b] = at
                for j0, nch in TILES0 if b == 0 else TILES:
                    st = spool.tile([L, 3 * SLOT], F32, tag="s")
                    for m in range(nch):
                        jj = j0 + m
                        r, c = divmod(jj, 2)
                        nc.tensor.matmul(
                            st[:, m * SLOT : m * SLOT + CH],
                            lhsT=kt[:, i * 672 + r * L : i * 672 + (r + 1) * L],
                            rhs=qt[:, i * 672 + c * CH : i * 672 + (c + 1) * CH],
                            start=True,
                            stop=True,
                        )
                    src = st[:, 0 : nch * SLOT].rearrange(
                        "s (a c) -> s a c", c=SLOT
                    )[:, :, 0:CH]
                    dst = at[:, j0 * CH : (j0 + nch) * CH].rearrange(
                        "s (a c) -> s a c", c=CH
                    )
                    nc.scalar.activation(dst, src, mybir.ActivationFunctionType.Exp)

            def stage2(b):
                """e2 accumulation, evacuate, transpose, normalize."""
                g, i = divmod(b, G)
                vt = vg[g]
                at = A[b]
                a3 = at[:].rearrange("s (r f) -> s r f", f=P * L)
                et = e2pool.tile([EP, 2 * SLOT], F32, tag="e2")
                Et[b] = et
                o0 = et[:, 0:384].rearrange("e (r l) -> e r l", l=L)
                o1 = et[:, SLOT : SLOT + 288].rearrange("e (r l) -> e r l", l=L)
                for p in range(P):
                    lhsT = vt[:, i * 77 + p * EP : i * 77 + (p + 1) * EP]
                    nc.tensor.matmul(
                        o0,
                        lhsT=lhsT,
                        rhs=a3[:, 0:4, p * L : (p + 1) * L],
                        start=(p == 0),
                        stop=(p == P - 1),
                    )
                    nc.tensor.matmul(
                        o1,
                        lhsT=lhsT,
                        rhs=a3[:, 4:7, p * L : (p + 1) * L],
                        start=(p == 0),
                        stop=(p == P - 1),
                    )

            def stage3(b):
                g, i = divmod(b, G)
                et = Et[b]
                eb = epool.tile([EP, P * L], F32, tag="eb")
                Eb[b] = eb
                nc.vector.tensor_copy(eb[:, 0:384], et[:, 0:384])
                nc.vector.tensor_copy(eb[:, 384:672], et[:, SLOT : SLOT + 288])
                if b >= bpc - 2:
                    # scores pool is idle for the final batches; keep the
                    # e2 accumulator slot free so the last e2 starts sooner
                    tt = spool.tile([L, P * EP], F32, tag="s")
                else:
                    tt = e2pool.tile([L, P * EP], F32, tag="e2")
                for r in range(P):
                    nc.tensor.transpose(
                        tt[:, r * EP : (r + 1) * EP],
                        eb[:, r * L : (r + 1) * L],
                        ident[:],
                    )
                t3 = tt[:].rearrange("l (r e) -> l r e", e=EP)
                rd = rdpool.tile([L, P], F32, tag="rd")
                r3 = rd[:].rearrange("l (r u) -> l r u", u=1)
                nc.vector.reciprocal(r3, t3[:, :, E : E + 1])
                if OG[g] is None:
                    og_tile = ogpool.tile([L, G * R], F32, tag="og")
                    OG[g] = og_tile
                og = OG[g]
                dst = og[:, i * R : (i + 1) * R].rearrange("l (e r) -> l r e", r=P)
                rdb = r3.copy()
                rdb.ap = rdb.ap[:-1] + [[0, E]]
                nc.vector.tensor_mul(dst, t3[:, :, 0:E], rdb)

            def flush_group(g):
                g0 = g * G
                if g == ngroups - 1:
                    # split the final flush so the tail only waits on the
                    # last batch's slice
                    nc.sync.dma_start(
                        o_d[g0 : g0 + G - 1].rearrange("b l c -> l b c"),
                        OG[g][:, 0 : (G - 1) * R].rearrange(
                            "l (b c) -> l b c", b=G - 1
                        ),
                    )
                    nc.sync.dma_start(
                        o_d[g0 + G - 1], OG[g][:, (G - 1) * R : G * R]
                    )
                else:
                    nc.sync.dma_start(
                        o_d[g0 : g0 + G].rearrange("b l c -> l b c"),
                        OG[g][:].rearrange("l (b c) -> l b c", b=G),
                    )
                OG[g] = None

            # software pipeline: stage1(b) ahead, stage2/3 one batch behind
            for rep in range(repeat):
                load_group(0)
                for b in range(bpc + 1):
                    if b < bpc:
                        g, i = divmod(b, G)
                        if i == 0 and g + 1 < ngroups:
                            load_group(g + 1)
                        stage1(b)
                    if b >= 1:
                        stage2(b - 1)
                        stage3(b - 1)
                        if (b - 1) % G == G - 1:
                            flush_group((b - 1) // G)

    nc.compile()
    return nc


def _get_nc(bpc=BPC, repeat=1):
    key = (bpc, repeat)
    if key not in _CACHE:
        _CACHE[key] = _build(bpc, repeat)
    return _CACHE[key]


def _prep(queries, keys, values):
    q = np.asarray(queries, dtype=np.float32)
    k = np.asarray(keys, dtype=np.float32)
    v = np.asarray(values, dtype=np.float32)
    b = q.shape[0]
    # Q2[b, e, p*96+l] = q[b, l, e*7+p]
    q2 = np.ascontiguousarray(
        q.reshape(b, L, E, P).transpose(0, 2, 3, 1).reshape(b, E, P * L)
    )
    # KT[b, e, r*96+s] = k[b, s, e*7+r]
    kt = np.ascontiguousarray(
        k.reshape(b, L, E, P).transpose(0, 2, 3, 1).reshape(b, E, P * L)
    )
    # VT[b, s, p*11+e'] = v[b, s, e'*7+p] for e'<10, 1.0 at e'=10
    v4 = v.reshape(b, L, E, P).transpose(0, 1, 3, 2)  # [b, s, p, e]
    vt = np.concatenate([v4, np.ones((b, L, P, 1), np.float32)], axis=-1)
    vt = np.ascontiguousarray(vt.reshape(b, L, P * EP))
    return q2, kt, vt


def kernel(queries, keys, values, attn_mask=None, _trace=False):
    nc = _get_nc()
    q2, kt, vt = _prep(queries, keys, values)
    in_maps = []
    for c in range(NCORES):
        s = slice(c * BPC, (c + 1) * BPC)
        in_maps.append({"q2": q2[s], "kt": kt[s], "vt": vt[s]})
    res = None
    for attempt in range(3):
        try:
            res = run_bass_kernel_spmd(
                nc, in_maps, core_ids=list(range(NCORES)), trace=_trace
            )
            break
        except Exception:
            # shared terminal occasionally reports transient NRT device
            # errors; back off and retry
            if attempt == 2:
                raise
            import time as _time

            _time.sleep(15)
    out = np.concatenate([res.results[c]["out"] for c in range(NCORES)], axis=0)
    if _trace:
        kernel.last_exec_time_ns = res.exec_time_ns
        kernel.last_results = res
    return out.astype(np.float32)


# revision 34
# speedup vs baseline: 1.0571x; 1.0367x over previous
"""Grouped-channel attention (CAT FullAttention) Trainium2 kernel.

Math (per batch element b; L=S=96, R=70, E=10, P=7):
  scores[l,s,p,r] = sum_e q[l,e,p] * k[s,e,r]
  A = softmax over (s,p) of scores           (per l, r)
  out[l,e,r]      = sum_{s,p} v[s,e,p] * A[l,s,p,r]

Strategy: pure data parallel over the batch dim (B=256 -> 32 per core x 8
cores). Per batch element on-device:
  e1   (PE) : per r, scores[s,(p,l)] = K_r^T @ Q2, fp32r matmuls, N=336
              chunks written to 512-element-aligned PSUM slots.
  exp  (ACT): exp over 3 PSUM slots per instr -> A [96, 4704] in SBUF.
  e2   (PE) : per p, E[e',(r,l)] += V_p^T @ A_p accumulated over p in PSUM.
              V carries a ones-channel at e'=10, so E[10,:] is the softmax
              denominator.
  tail (PE/DVE): transpose E to [l,(r,e')], reciprocal of the denominator,
              broadcast multiply, contiguous DMA of [96,70] per batch.
"""

import sys

if "/opt/trn_rl_repo" not in sys.path:
    sys.path.insert(0, "/opt/trn_rl_repo")

import numpy as np

import concourse.bass as bass
import concourse.bacc as bacc
import concourse.tile as tile
from concourse import mybir, masks
from concourse.bass_utils import run_bass_kernel_spmd

B, L, R = 256, 96, 70
E, P = 10, 7
EP = E + 1  # v channels + ones channel
NCORES = 8
BPC = B // NCORES  # batches per core
G = 4  # batches per DMA group
F32R = mybir.dt.float32r
F32 = mybir.dt.float32

_CACHE = {}


def _build(bpc, repeat=1):
    nc = bacc.Bacc("TRN2", target_bir_lowering=False, debug=False, num_devices=NCORES)
    q_d = nc.dram_tensor("q2", [bpc, E, P * L], F32R, kind="ExternalInput").ap()
    k_d = nc.dram_tensor("kt", [bpc, E, P * L], F32R, kind="ExternalInput").ap()
    v_d = nc.dram_tensor("vt", [bpc, L, P * EP], F32R, kind="ExternalInput").ap()
    o_d = nc.dram_tensor("out", [bpc, L, R], F32, kind="ExternalOutput").ap()

    ngroups = bpc // G
    CH = 336  # e1 chunk width: (p,l)=672 split in two, each >=256 for fp32r
    SLOT = 512  # psum chunk slot (one bank)
    NCHUNK = 2 * P  # 14 chunks of 336 per batch
    # chunk-tile packing: 3 slots per scores tile -> 5 ACT instrs per batch
    TILES = [(0, 3), (3, 3), (6, 3), (9, 2), (11, 3)]
    # batch 0 leads with a 1-slot region so the first exp fires one cold
    # matmul after the DMA instead of three
    TILES0 = [(0, 1), (1, 2), (3, 3), (6, 3), (9, 2), (11, 3)]

    with tile.TileContext(nc) as tc:
        with (
            tc.tile_pool(name="const", bufs=1) as cpool,
            tc.tile_pool(name="qk", bufs=2) as qkpool,
            tc.tile_pool(name="apool", bufs=3) as apool,
            tc.tile_pool(name="esb", bufs=2) as epool,
            tc.tile_pool(name="rd", bufs=2) as rdpool,
            tc.tile_pool(name="og", bufs=2) as ogpool,
            tc.tile_pool(name="ps_s", bufs=2, space="PSUM") as spool,
            tc.tile_pool(name="ps_e", bufs=1, space="PSUM") as e2pool,
        ):
            ident = cpool.tile([EP, EP], F32)
            masks.make_identity(nc, ident[:])
            # touch Exp once so the ACT table set loads during the first DMA
            warm = cpool.tile([1, 1], F32)
            nc.scalar.activation(
                warm[:], ident[0:1, 0:1], mybir.ActivationFunctionType.Exp
            )

            # per-group input tiles, per-batch A tiles
            qg = [None] * ngroups
            kg = [None] * ngroups
            vg = [None] * ngroups
            A = [None] * bpc
            Et = [None] * bpc
            Eb = [None] * bpc
            OG = [None] * ngroups

            def load_group(g):
                qt = qkpool.tile([E, G * P * L], F32R, tag="qg")
                kt = qkpool.tile([E, G * P * L], F32R, tag="kg")
                vt = qkpool.tile([L, G * P * EP], F32R, tag="vg")
                g0 = g * G
                if g == 0:
                    # split the first group's q/k so batch 0 is ready early
                    nc.sync.dma_start(qt[:, 0:672], q_d[g0])
                    nc.sync.dma_start(kt[:, 0:672], k_d[g0])
                    nc.sync.dma_start(
                        qt[:, 672:].rearrange("e (b f) -> e b f", b=G - 1),
                        q_d[g0 + 1 : g0 + G].rearrange("b e f -> e b f"),
                    )
                    nc.sync.dma_start(
                        kt[:, 672:].rearrange("e (b f) -> e b f", b=G - 1),
                        k_d[g0 + 1 : g0 + G].rearrange("b e f -> e b f"),
                    )
                else:
                    nc.sync.dma_start(
                        qt[:].rearrange("e (b f) -> e b f", b=G),
                        q_d[g0 : g0 + G].rearrange("b e f -> e b f"),
                    )
                    nc.sync.dma_start(
                        kt[:].rearrange("e (b f) -> e b f", b=G),
                        k_d[g0 : g0 + G].rearrange("b e f -> e b f"),
                    )
                nc.sync.dma_start(
                    vt[:].rearrange("s (b f) -> s b f", b=G),
                    v_d[g0 : g0 + G].rearrange("b s f -> s b f"),
                )
                qg[g], kg[g], vg[g] = qt, kt, vt

            def stage1(b):
                """e1 matmuls + exp into A[b]."""
                g, i = divmod(b, G)
                qt, kt = qg[g], kg[g]
                at = apool.tile([L, P * P * L], F32R, tag="A")
                A[b] = at
                for j0, nch in TILES0 if b == 0 else TILES:
                    st = spool.tile([L, 3 * SLOT], F32, tag="s")
                    for m in range(nch):
                        jj = j0 + m
                        r, c = divmod(jj, 2)
                        nc.tensor.matmul(
                            st[:, m * SLOT : m * SLOT + CH],
                            lhsT=kt[:, i * 672 + r * L : i * 672 + (r + 1) * L],
                            rhs=qt[:, i * 672 + c * CH : i * 672 + (c + 1) * CH],
                            start=True,
                            stop=True,
                        )
                    src = st[:, 0 : nch * SLOT].rearrange(
                        "s (a c) -> s a c", c=SLOT
                    )[:, :, 0:CH]
                    dst = at[:, j0 * CH : (j0 + nch) * CH].rearrange(
                        "s (a c) -> s a c", c=CH
                    )
                    nc.scalar.activation(dst, src, mybir.ActivationFunctionType.Exp)

            def stage2(b):
                """e2 accumulation, evacuate, transpose, normalize."""
                g, i = divmod(b, G)
                vt = vg[g]
                at = A[b]
                a3 = at[:].rearrange("s (r f) -> s r f", f=P * L)
                # two separate 1-bank accumulators so the r0-3 group (whose
                # exp inputs finish 2 regions earlier) runs and evacuates
                # while the r4-6 group still waits on the final exps
                et0 = e2pool.tile([EP, 384], F32, tag="e2a")
                et1 = e2pool.tile([EP, 288], F32, tag="e2b")
                Et[b] = (et0, et1)
                o0 = et0[:].rearrange("e (r l) -> e r l", l=L)
                o1 = et1[:].rearrange("e (r l) -> e r l", l=L)
                for p in range(P):
                    nc.tensor.matmul(
                        o0,
                        lhsT=vt[:, i * 77 + p * EP : i * 77 + (p + 1) * EP],
                        rhs=a3[:, 0:4, p * L : (p + 1) * L],
                        start=(p == 0),
                        stop=(p == P - 1),
                    )
                for p in range(P):
                    nc.tensor.matmul(
                        o1,
                        lhsT=vt[:, i * 77 + p * EP : i * 77 + (p + 1) * EP],
                        rhs=a3[:, 4:7, p * L : (p + 1) * L],
                        start=(p == 0),
                        stop=(p == P - 1),
                    )

            def stage3(b):
                g, i = divmod(b, G)
                et = Et[b]

                eb = epool.tile([EP, P * L], F32, tag="eb")
                Eb[b] = eb
                et0, et1 = et
                nc.vector.tensor_copy(eb[:, 0:384], et0[:])
                nc.vector.tensor_copy(eb[:, 384:672], et1[:])
                if b >= bpc - 2:
                    # scores pool is idle for the final batches; keep the
                    # e2 accumulator slot free so the last e2 starts sooner
                    tt = spool.tile([L, P * EP], F32, tag="s")
                else:
                    tt = e2pool.tile([L, P * EP], F32, tag="e2a")
                for r in range(P):
                    nc.tensor.transpose(
                        tt[:, r * EP : (r + 1) * EP],
                        eb[:, r * L : (r + 1) * L],
                        ident[:],
                    )
                t3 = tt[:].rearrange("l (r e) -> l r e", e=EP)
                rd = rdpool.tile([L, P], F32, tag="rd")
                r3 = rd[:].rearrange("l (r u) -> l r u", u=1)
                nc.vector.reciprocal(r3, t3[:, :, E : E + 1])
                if OG[g] is None:
                    og_tile = ogpool.tile([L, G * R], F32, tag="og")
                    OG[g] = og_tile
                og = OG[g]
                dst = og[:, i * R : (i + 1) * R].rearrange("l (e r) -> l r e", r=P)
                rdb = r3.copy()
                rdb.ap = rdb.ap[:-1] + [[0, E]]
                nc.vector.tensor_mul(dst, t3[:, :, 0:E], rdb)

            def flush_group(g):
                g0 = g * G
                if g == ngroups - 1:
                    # split the final flush so the tail only waits on the
                    # last batch's slice
                    nc.sync.dma_start(
                        o_d[g0 : g0 + G - 1].rearrange("b l c -> l b c"),
                        OG[g][:, 0 : (G - 1) * R].rearrange(
                            "l (b c) -> l b c", b=G - 1
                        ),
                    )
                    nc.sync.dma_start(
                        o_d[g0 + G - 1], OG[g][:, (G - 1) * R : G * R]
                    )
                else:
                    nc.sync.dma_start(
                        o_d[g0 : g0 + G].rearrange("b l c -> l b c"),
                        OG[g][:].rearrange("l (b c) -> l b c", b=G),
                    )
                OG[g] = None

            # software pipeline: stage1(b) ahead, stage2/3 one batch behind
            for rep in range(repeat):
                load_group(0)
                for b in range(bpc + 1):
                    if b < bpc:
                        g, i = divmod(b, G)
                        if i == 0 and g + 1 < ngroups:
                            load_group(g + 1)
                        stage1(b)
                    if b >= 1:
                        stage2(b - 1)
                        stage3(b - 1)
                        if (b - 1) % G == G - 1:
                            flush_group((b - 1) // G)

    nc.compile()
    return nc


def _get_nc(bpc=BPC, repeat=1):
    key = (bpc, repeat)
    if key not in _CACHE:
        _CACHE[key] = _build(bpc, repeat)
    return _CACHE[key]


def _prep(queries, keys, values):
    q = np.asarray(queries, dtype=np.float32)
    k = np.asarray(keys, dtype=np.float32)
    v = np.asarray(values, dtype=np.float32)
    b = q.shape[0]
    # Q2[b, e, p*96+l] = q[b, l, e*7+p]
    q2 = np.ascontiguousarray(
        q.reshape(b, L, E, P).transpose(0, 2, 3, 1).reshape(b, E, P * L)
    )
    # KT[b, e, r*96+s] = k[b, s, e*7+r]
    kt = np.ascontiguousarray(
        k.reshape(b, L, E, P).transpose(0, 2, 3, 1).reshape(b, E, P * L)
    )
    # VT[b, s, p*11+e'] = v[b, s, e'*7+p] for e'<10, 1.0 at e'=10
    v4 = v.reshape(b, L, E, P).transpose(0, 1, 3, 2)  # [b, s, p, e]
    vt = np.concatenate([v4, np.ones((b, L, P, 1), np.float32)], axis=-1)
    vt = np.ascontiguousarray(vt.reshape(b, L, P * EP))
    return q2, kt, vt


def kernel(queries, keys, values, attn_mask=None, _trace=False):
    nc = _get_nc()
    q2, kt, vt = _prep(queries, keys, values)
    in_maps = []
    for c in range(NCORES):
        s = slice(c * BPC, (c + 1) * BPC)
        in_maps.append({"q2": q2[s], "kt": kt[s], "vt": vt[s]})
    res = None
    for attempt in range(3):
        try:
            res = run_bass_kernel_spmd(
                nc, in_maps, core_ids=list(range(NCORES)), trace=_trace
            )
            break
        except Exception:
            # shared terminal occasionally reports transient NRT device
            # errors; back off and retry
            if attempt == 2:
                raise
            import time as _time

            _time.sleep(15)
    out = np.concatenate([res.results[c]["out"] for c in range(NCORES)], axis=0)
    if _trace:
        kernel.last_exec_time_ns = res.exec_time_ns
        kernel.last_results = res
    return out.astype(np.float32)
